# revision 1
# baseline (speedup 1.0000x reference)
"""Trainium2 Bass kernel for nn_AllToAllMoE (degenerate single-group MoE == dense MLP).

reference:  y = gelu(x @ w1 + b1, exact) @ w2 + b2
  x  (16384, 2048) f32
  w1 (2048, 8192) f32, b1 (8192,) f32
  w2 (8192, 2048) f32, b2 (2048,) f32

Strategy: the all_to_all with a single replica group is an identity permutation,
so the problem is a dense 2-layer MLP. We shard TOKENS across the 8 NeuronCores
(data parallel — zero collectives needed; outputs are disjoint row blocks).

Per-core layout trick: everything is computed in the "transposed" orientation so
that every DMA is natural row-major:
  GEMM1: hT = (w1)^T-stationary:  matmul(out=hT[f,:], lhsT=w1[k,f], rhs=xT[k,t])
         -> hT tile has FFN on partitions, tokens on free dim.
  GELU+b1 on ScalarE with per-partition bias (b1 pre-transposed on host).
  GEMM2: yT = matmul(out=yT[n,:], lhsT=w2[k2,n], rhs=hT[k2,t])
         -> yT has HIDDEN on partitions; DMA'd out natural-row-major; host
            transposes the (hidden, tokens) shard back.
The only data transposes (x shard, b1, b2, y result) happen on the host.

Compute dtype: bf16 operands with fp32 PSUM accumulation (1 PE cycle/row — the
same TensorEngine rate as fp32r, 4x faster than plain fp32), well within the
rel-err budget for randn-scaled data.
"""

import os
import numpy as np
import ml_dtypes

import concourse.bass as bass
import concourse.mybir as mybir
import concourse.tile as tile
from concourse import bacc
from concourse.bass_utils import run_bass_kernel_spmd

P = 128
N_CORES = 8

TOKENS = 16384
HIDDEN = 2048
FFN = 8192

BF16 = mybir.dt.bfloat16
F32 = mybir.dt.float32

NP_BF16 = np.dtype(ml_dtypes.bfloat16)


def build_mlp(T, H, F, TB, n_cores=N_CORES, w1_bufs=6, w2_bufs=5, xt_bufs=1,
              psum_bufs=4, y_bufs=4, xt_splits=4):
    """Build the per-core fused MLP graph (SPMD: same graph on all cores).

    T: tokens per core, H: hidden, F: ffn, TB: token block (free dim of all MMs).
    """
    assert T % TB == 0 and H % P == 0 and F % P == 0
    KH = H // P          # contraction tiles for GEMM1
    KF = F // P          # contraction tiles for GEMM2
    FT = F // P          # ffn output tiles (phase A)
    HT = H // P          # hidden output tiles (phase B)
    assert FT % 2 == 0 and HT % 2 == 0 and KF % KH == 0
    n_blocks = T // TB
    W2G = KF // KH       # w2 k-chunks per n-pair

    nc = bacc.Bacc("TRN2", target_bir_lowering=False, debug=False,
                   num_devices=n_cores)

    # b1t/b2t are packed into one (P, FT+HT+pad) f32 tensor with 128-col
    # (512B) rows so the bias DMA takes the full-rate path, not sub-512B RMW.
    BCOLS = -(-(FT + HT) // P) * P
    xT_d = nc.dram_tensor("xT", (H, T), BF16, kind="ExternalInput").ap()
    w1_d = nc.dram_tensor("w1", (H, F), BF16, kind="ExternalInput").ap()
    w2_d = nc.dram_tensor("w2", (F, H), BF16, kind="ExternalInput").ap()
    bc_d = nc.dram_tensor("bc", (P, BCOLS), F32, kind="ExternalInput").ap()
    out_d = nc.dram_tensor("out", (H, T), F32, kind="ExternalOutput").ap()

    xT_r = xT_d.rearrange("(k p) t -> p k t", p=P)
    w1_r = w1_d.rearrange("(k p) f -> p k f", p=P)
    w2_r = w2_d.rearrange("(k p) h -> p k h", p=P)

    GELU = mybir.ActivationFunctionType.Gelu
    IDENT = mybir.ActivationFunctionType.Identity

    with tile.TileContext(nc) as tc:
        with (
            tc.tile_pool(name="const", bufs=1) as const_pool,
            tc.tile_pool(name="xt", bufs=xt_bufs) as xt_pool,
            tc.tile_pool(name="w1", bufs=w1_bufs) as w1_pool,
            tc.tile_pool(name="w2", bufs=w2_bufs) as w2_pool,
            tc.tile_pool(name="ht", bufs=1) as ht_pool,
            tc.tile_pool(name="y", bufs=y_bufs) as y_pool,
            tc.tile_pool(name="psA", bufs=psum_bufs, space="PSUM") as psA,
            tc.tile_pool(name="psB", bufs=psum_bufs, space="PSUM") as psB,
        ):
            bc = const_pool.tile([P, BCOLS], F32)
            b1t = bc[:, 0:FT]
            b2t = bc[:, FT:FT + HT]

            hT = ht_pool.tile([P, FT, TB], BF16)

            w1s0 = None
            for t in range(n_blocks):
                ts_ = slice(t * TB, (t + 1) * TB)
                xt = xt_pool.tile([P, KH, TB], BF16)
                xt_splits = min(xt_splits, KH)
                ksp = KH // xt_splits
                if t == 0:
                    # cold-start ramp: DMA descriptor setups serialize
                    # (~0.65us each) on the issuing sequencer, and the first
                    # MM only needs w1-chunk0 + xt-chunk0 — interleave
                    # k-granular chunks of both, and load biases after (they
                    # are first needed ~4us later by the first activation).
                    w1s0 = w1_pool.tile([P, KH, 2 * P], BF16, tag="w1s")
                    sp = min(4, KH)
                    kc = KH // sp
                    for c in range(max(sp, xt_splits)):
                        if c < sp:
                            nc.sync.dma_start(
                                out=w1s0[:, c * kc:(c + 1) * kc, :],
                                in_=w1_r[:, c * kc:(c + 1) * kc, 0:2 * P])
                        if c < xt_splits:
                            nc.sync.dma_start(
                                out=xt[:, c * ksp:(c + 1) * ksp, :],
                                in_=xT_r[:, c * ksp:(c + 1) * ksp, ts_])
                    nc.sync.dma_start(out=bc[:], in_=bc_d[:])
                else:
                    for c in range(xt_splits):
                        nc.sync.dma_start(
                            out=xt[:, c * ksp:(c + 1) * ksp, :],
                            in_=xT_r[:, c * ksp:(c + 1) * ksp, ts_])

                # ---- phase A: hT[f, tokens] = gelu(w1^T x^T + b1) ----
                for fp in range(FT // 2):
                    fsl = slice(fp * 2 * P, (fp + 1) * 2 * P)
                    if t == 0 and fp == 0:
                        w1s = w1s0
                    else:
                        w1s = w1_pool.tile([P, KH, 2 * P], BF16, tag="w1s")
                        nc.sync.dma_start(out=w1s[:], in_=w1_r[:, :, fsl])
                    for sub in range(2):
                        f = fp * 2 + sub
                        ps = psA.tile([P, TB], F32)
                        for k in range(KH):
                            nc.tensor.matmul(
                                ps[:],
                                lhsT=w1s[:, k, sub * P:(sub + 1) * P],
                                rhs=xt[:, k, :],
                                start=(k == 0), stop=(k == KH - 1))
                        nc.scalar.activation(
                            hT[:, f, :], ps[:], GELU, bias=b1t[:, f:f + 1])

                # ---- phase B: yT[n, tokens] = w2^T hT + b2 ----
                for npair in range(HT // 2):
                    w2cs = []
                    for g in range(W2G):
                        w2c = w2_pool.tile([P, KH, 2 * P], BF16)
                        nc.sync.dma_start(
                            out=w2c[:],
                            in_=w2_r[:, g * KH:(g + 1) * KH,
                                     npair * 2 * P:(npair + 1) * 2 * P])
                        w2cs.append(w2c)
                    for sub in range(2):
                        n = npair * 2 + sub
                        ps2 = psB.tile([P, TB], F32)
                        for k2 in range(KF):
                            g, kk = divmod(k2, KH)
                            nc.tensor.matmul(
                                ps2[:],
                                lhsT=w2cs[g][:, kk, sub * P:(sub + 1) * P],
                                rhs=hT[:, k2, :],
                                start=(k2 == 0), stop=(k2 == KF - 1))
                        y = y_pool.tile([P, TB], F32)
                        if t == n_blocks - 1 and n == HT - 1:
                            # drain the very last tile in halves so the final
                            # store overlaps the activation (kernel tail)
                            for hv in range(2):
                                hsl = slice(hv * TB // 2, (hv + 1) * TB // 2)
                                nc.scalar.activation(
                                    y[:, hsl], ps2[:, hsl], IDENT,
                                    bias=b2t[:, n:n + 1])
                                nc.sync.dma_start(
                                    out=out_d[n * P:(n + 1) * P,
                                              t * TB + hv * TB // 2:
                                              t * TB + (hv + 1) * TB // 2],
                                    in_=y[:, hsl])
                        else:
                            nc.scalar.activation(
                                y[:], ps2[:], IDENT, bias=b2t[:, n:n + 1])
                            nc.sync.dma_start(
                                out=out_d[n * P:(n + 1) * P, ts_], in_=y[:])

    nc.compile()
    return nc


def make_in_maps(x, w1, b1, w2, b2, n_cores=N_CORES):
    """Shard FULL f32 inputs into per-core in_maps (host-side layout prep)."""
    T_core = x.shape[0] // n_cores
    FT = w1.shape[1] // P
    HT = w2.shape[1] // P
    w1_b = np.ascontiguousarray(w1.astype(NP_BF16))
    w2_b = np.ascontiguousarray(w2.astype(NP_BF16))
    b1t = b1.astype(np.float32).reshape(FT, P).T
    b2t = b2.astype(np.float32).reshape(HT, P).T
    BCOLS = -(-(FT + HT) // P) * P
    bc = np.zeros((P, BCOLS), dtype=np.float32)
    bc[:, 0:FT] = b1t
    bc[:, FT:FT + HT] = b2t
    in_maps = []
    for i in range(n_cores):
        xs = x[i * T_core:(i + 1) * T_core]
        xT = np.ascontiguousarray(xs.T.astype(NP_BF16))
        in_maps.append({"xT": xT, "w1": w1_b, "w2": w2_b, "bc": bc})
    return in_maps


_CACHE = {}


def _get_nc():
    if "nc" not in _CACHE:
        _CACHE["nc"] = build_mlp(TOKENS // N_CORES, HIDDEN, FFN, TB=512)
    return _CACHE["nc"]


def run(x, w1, b1, w2, b2, trace=False, **kw):
    nc = _get_nc()
    in_maps = make_in_maps(x, w1, b1, w2, b2)
    res = run_bass_kernel_spmd(nc, in_maps, core_ids=list(range(N_CORES)),
                               trace=trace, **kw)
    T_core = x.shape[0] // N_CORES
    y = np.concatenate(
        [np.asarray(res.results[i]["out"]).T for i in range(N_CORES)], axis=0)
    return np.ascontiguousarray(y.astype(np.float32)), res


def kernel(x, w1, b1, w2, b2):
    x = np.asarray(x, dtype=np.float32)
    w1 = np.asarray(w1, dtype=np.float32)
    b1 = np.asarray(b1, dtype=np.float32)
    w2 = np.asarray(w2, dtype=np.float32)
    b2 = np.asarray(b2, dtype=np.float32)
    y, _ = run(x, w1, b1, w2, b2, trace=False)
    return y

